# revision 12
# baseline (speedup 1.0000x reference)
"""ChannelAtten (XCA-style channel attention) on 8 TRN2 NeuronCores. v8

Baseline structure with three surgical changes: bf16 qkv/dw tiles, v~ kept
in SBUF (no DRAM round trip; bf16 attn@v), and a 2-head-packed gram.

Sharding: (batch, H-half) -> 8 shards. Per core: fused qkv 1x1 conv (PE),
depthwise 3x3 via diagonal matmuls accumulated in PSUM (PE), PE-transpose of
q,k chunks feeding a persistent gram-PSUM accumulation, sumsq via fused
square+accum, a tiny pairwise AllReduce (gram + sumsq), on-chip softmax,
attn@v (block-diag lhsT) and the 1x1 proj, streamed back to HBM.
"""

import sys

sys.path.insert(0, "/opt/trn_rl_repo")

import numpy as np

import concourse.bass as bass
import concourse.mybir as mybir
import concourse.tile as tile
from concourse import bacc
from concourse.bass_utils import run_bass_kernel_spmd

F32 = mybir.dt.float32
BF16 = mybir.dt.bfloat16
AF = mybir.ActivationFunctionType
OP = mybir.AluOpType
AX = mybir.AxisListType
F32R = mybir.dt.float32r

NPBF16 = mybir.dt.np(BF16)


def r32(ap):
    return ap.bitcast(F32R)

DIM = 192
HEAD_DIM = 48
NH = 4
H = 256
W = 256
B = 4
N_CORES = 8
HALF = H // 2          # 128 output rows per shard
PR = HALF + 2          # 130 padded rows per shard
C3 = 3 * DIM           # 576
SCALE = HEAD_DIM ** -0.5
EPS = 1e-12

# channel tiles over the 576 qkv channels
CT = [(0, 128), (128, 128), (256, 128), (384, 128), (512, 64)]
NT = len(CT)
NITER = PR // 2        # 65 qkv row-pair iterations
NDW = NITER - 1        # 64 dw/output row-pair iterations

_CACHED = {}


def _build_nc(repeat=1, no_cc=False):
    nc = bacc.Bacc("TRN2", target_bir_lowering=False, debug=False,
                   enable_asserts=True, num_devices=N_CORES)

    x_d = nc.dram_tensor("x_sh", [DIM, PR, W], F32, kind="ExternalInput").ap()
    wqkvT_d = nc.dram_tensor("wqkvT", [DIM, C3], F32, kind="ExternalInput").ap()
    wdw_d = nc.dram_tensor("wdw_diag", [128, 9 * NT * 128], BF16,
                           kind="ExternalInput").ap()
    wprojT_d = nc.dram_tensor("wprojT", [DIM, DIM], F32, kind="ExternalInput").ap()
    ident_d = nc.dram_tensor("ident", [128, 128], F32, kind="ExternalInput").ap()
    bias_d = nc.dram_tensor("bias_pack", [32, 128], F32, kind="ExternalInput").ap()
    out_d = nc.dram_tensor("out_sh", [DIM, HALF, W], F32, kind="ExternalOutput").ap()

    with tile.TileContext(nc) as tc:
        for _rep in range(repeat):
            with (
                tc.tile_pool(name="const", bufs=1) as constp,
                tc.tile_pool(name="xin", bufs=2) as xpool,
                tc.tile_pool(name="qkv", bufs=3) as qkvpool,
                tc.tile_pool(name="dwsb", bufs=2) as dwsb,
                tc.tile_pool(name="sT", bufs=3) as sTpool,
                tc.tile_pool(name="trash", bufs=2) as trashp,
                tc.tile_pool(name="small", bufs=1) as smallp,
                tc.tile_pool(name="aosb", bufs=2) as aosb,
                tc.tile_pool(name="outsb", bufs=2) as outsbp,
                tc.tile_pool(name="dram", bufs=1, space="DRAM") as dram,
            ):
                # ---- constants into SBUF ----
                wq_a = constp.tile([128, C3], F32)   # wqkvT rows 0:128
                wq_b = constp.tile([64, C3], F32)    # rows 128:192
                nc.sync.dma_start(r32(wq_a[:]), r32(wqkvT_d[0:128, :]))
                nc.sync.dma_start(r32(wq_b[:]), r32(wqkvT_d[128:192, :]))
                wdw_sb = constp.tile([128, 9 * NT * 128], BF16)
                nc.sync.dma_start(wdw_sb[:], wdw_d[:])
                wp_a = constp.tile([96, DIM], F32)   # wprojT rows 0:96
                wp_b = constp.tile([96, DIM], F32)   # rows 96:192
                nc.sync.dma_start(r32(wp_a[:]), r32(wprojT_d[0:96, :]))
                nc.sync.dma_start(r32(wp_b[:]), r32(wprojT_d[96:192, :]))
                ident = constp.tile([128, 128], F32)
                nc.sync.dma_start(ident[:], ident_d[:])
                ident_bf = constp.tile([128, 128], BF16)
                nc.scalar.copy(ident_bf[:], ident[:])
                bias_sb = constp.tile([128, 32], F32)
                nc.sync.dma_start(bias_sb[:], bias_d.rearrange("r c -> c r"))

                def bias_col(kind, t):
                    # kind: 0=bqkv 1=bdw 2=hbtop 3=hbbot 4=bproj
                    return bias_sb[0:CT[t][1] if kind < 4 else (128 if t == 0 else 64),
                                   5 * kind + t : 5 * kind + t + 1]

                ssq_parts = constp.tile([128, 3 * NDW], F32)
                vA = constp.tile([96, NDW, 2, W], BF16)   # v channels 0:96
                vB = constp.tile([96, NDW, 2, W], BF16)   # v channels 96:192
                g_bin = dram.tile([48, 4 * 48], F32)
                g_bout = dram.tile([48, 4 * 48], F32)
                s_bin = dram.tile([128, 3], F32)
                s_bout = dram.tile([128, 3], F32)

                with (
                    tc.tile_pool(name="qkps", bufs=2, space="PSUM") as qkps,
                    tc.tile_pool(name="dwps", bufs=2, space="PSUM") as dwps,
                    tc.tile_pool(name="tps", bufs=3, space="PSUM") as tps,
                    tc.tile_pool(name="gram", bufs=1, space="PSUM") as gramp,
                ):
                    gram_ps = gramp.tile([96, 2 * 96], F32)
                    qkv_prev = None  # tiles for row-pair j-1
                    first_gram = [True]

                    for j in range(NITER):
                        # ---- load x rows p=2j,2j+1; qkv conv ----
                        xa = xpool.tile([128, 2, W], F32, tag="xa")
                        xb = xpool.tile([64, 2, W], F32, tag="xb")
                        nc.sync.dma_start(r32(xa[:]), r32(x_d[0:128, 2 * j : 2 * j + 2, :]))
                        nc.sync.dma_start(r32(xb[:]), r32(x_d[128:192, 2 * j : 2 * j + 2, :]))
                        qkv_cur = []
                        for t, (c0, nt) in enumerate(CT):
                            ps = qkps.tile([128, 2, W], F32, tag="qkps")
                            nc.tensor.matmul(ps[0:nt], r32(wq_a[:, c0 : c0 + nt]),
                                             r32(xa[:]), start=True, stop=False)
                            nc.tensor.matmul(ps[0:nt], r32(wq_b[:, c0 : c0 + nt]),
                                             r32(xb[:]), start=False, stop=True)
                            sb = qkvpool.tile([128, 2, W + 2], BF16, tag=f"qkv{t}")
                            if j < 3:  # zero the pad cols once per pool slot
                                nc.gpsimd.memset(sb[0:nt, :, 0:1], 0.0)
                                nc.gpsimd.memset(sb[0:nt, :, W + 1 : W + 2], 0.0)
                            if j == 0:
                                nc.scalar.activation(sb[0:nt, 0, 1 : W + 1],
                                                     ps[0:nt, 0],
                                                     AF.Identity, bias=bias_col(2, t))
                                nc.scalar.activation(sb[0:nt, 1, 1 : W + 1],
                                                     ps[0:nt, 1],
                                                     AF.Identity, bias=bias_col(0, t))
                            elif j == NITER - 1:
                                nc.scalar.activation(sb[0:nt, 0, 1 : W + 1],
                                                     ps[0:nt, 0],
                                                     AF.Identity, bias=bias_col(0, t))
                                nc.scalar.activation(sb[0:nt, 1, 1 : W + 1],
                                                     ps[0:nt, 1],
                                                     AF.Identity, bias=bias_col(3, t))
                            else:
                                nc.scalar.activation(sb[0:nt, :, 1 : W + 1],
                                                     ps[0:nt],
                                                     AF.Identity, bias=bias_col(0, t))
                            qkv_cur.append(sb)

                        if j >= 1:
                            i = j - 1  # output rows 2i, 2i+1
                            A, Bt = qkv_prev, qkv_cur
                            dwq = []
                            for t, (c0, nt) in enumerate(CT):
                                dps = dwps.tile([128, 2, W], F32, tag="dwps")

                                def dg(kh, kw):
                                    base = ((kh * 3 + kw) * NT + t) * 128
                                    return wdw_sb[0:nt, base : base + nt]

                                # kh=0 rows (2i,2i+1) -> prev tile rows 0:2
                                pe_kw0 = (1, 0, 2) if t >= 3 else (1,)
                                for kw in pe_kw0:
                                    nc.tensor.matmul(dps[0:nt, 0:2, :], dg(0, kw),
                                                     A[t][0:nt, 0:2, kw : kw + W],
                                                     start=(kw == 1), stop=False)
                                if t < 3:
                                    # taps (0,0),(0,2) on DVE; b_dw folded into tap1
                                    acc = dwsb.tile([128, 2, W], BF16, tag="dwacc")
                                    nc.vector.tensor_scalar(
                                        acc[0:nt], A[t][0:nt, 0:2, 0:W],
                                        bias_sb[0:nt, 22 + t : 23 + t],
                                        bias_col(1, t), op0=OP.mult, op1=OP.add)
                                    nc.vector.scalar_tensor_tensor(
                                        acc[0:nt], A[t][0:nt, 0:2, 2 : 2 + W],
                                        bias_sb[0:nt, 27 + t : 28 + t], acc[0:nt],
                                        op0=OP.mult, op1=OP.add)
                                # kh=1: out row0 <- prev row1 ; out row1 <- cur row0
                                for kw in (0, 1, 2):
                                    nc.tensor.matmul(dps[0:nt, 0, :], dg(1, kw),
                                                     A[t][0:nt, 1, kw : kw + W],
                                                     start=False, stop=False)
                                    nc.tensor.matmul(dps[0:nt, 1, :], dg(1, kw),
                                                     Bt[t][0:nt, 0, kw : kw + W],
                                                     start=False, stop=False)
                                # kh=2: out rows (2i,2i+1) <- cur rows 0:2
                                for kw in (0, 1, 2):
                                    nc.tensor.matmul(dps[0:nt, 0:2, :], dg(2, kw),
                                                     Bt[t][0:nt, 0:2, kw : kw + W],
                                                     start=False, stop=(kw == 2))
                                if t < 3:
                                    sb2 = dwsb.tile([128, 2, W], BF16, tag=f"dw{t}")
                                    nc.vector.scalar_tensor_tensor(
                                        sb2[0:nt], dps[0:nt], 1.0, acc[0:nt],
                                        op0=OP.mult, op1=OP.add)
                                    dwq.append(sb2)
                                elif t == 3:  # v ch 0..127 -> vA + vB[0:32]
                                    nc.scalar.activation(vA[:, i], dps[0:96],
                                                         AF.Identity,
                                                         bias=bias_sb[0:96, 8:9])
                                    nc.scalar.activation(vB[0:32, i], dps[96:128],
                                                         AF.Identity,
                                                         bias=bias_sb[96:128, 5 + t : 6 + t])
                                else:  # v ch 128..191 -> vB[32:96]
                                    nc.scalar.activation(vB[32:64, i], dps[0:32],
                                                         AF.Identity,
                                                         bias=bias_sb[0:32, 9:10])
                                    nc.scalar.activation(vB[64:96, i], dps[32:64],
                                                         AF.Identity,
                                                         bias=bias_sb[32:64, 9:10])

                            # sumsq for q,k: ACT square + per-partition accum
                            for t in range(3):
                                tr = trashp.tile([128, 2, W], BF16, tag="trash")
                                nc.scalar.activation(
                                    tr[:], dwq[t][:], AF.Square,
                                    accum_out=ssq_parts[:, NDW * t + i : NDW * t + i + 1])

                            # transposes + gram
                            for s in range(4):
                                row, hf = divmod(s, 2)
                                sT = sTpool.tile([128, 384], BF16, tag="sT")
                                for t in range(3):
                                    tp = tps.tile([128, 128], BF16, tag="tps")
                                    nc.tensor.transpose(
                                        tp[:], dwq[t][:, row, 128 * hf : 128 * hf + 128],
                                        ident_bf[:])
                                    # split PSUM evacs across DVE and ACT
                                    if (s * 3 + t) % 2 == 0:
                                        nc.vector.tensor_copy(
                                            sT[:, 128 * t : 128 * t + 128], tp[:])
                                    else:
                                        nc.scalar.copy(
                                            sT[:, 128 * t : 128 * t + 128], tp[:])
                                last = (i == NDW - 1 and s == 3)
                                nc.tensor.matmul(
                                    gram_ps[:, 0:96],
                                    sT[:, 0:96], sT[:, 192:288],
                                    start=first_gram[0], stop=last,
                                    skip_group_check=True)
                                nc.tensor.matmul(
                                    gram_ps[:, 96:192],
                                    sT[:, 96:192], sT[:, 288:384],
                                    start=first_gram[0], stop=last,
                                    skip_group_check=True)
                                first_gram[0] = False
                        qkv_prev = qkv_cur

                    # ---- phase B: norms + collective + softmax ----
                    gfull = smallp.tile([96, 2 * 96], F32)
                    nc.vector.tensor_copy(gfull[:], gram_ps[:])
                    gram_sb = smallp.tile([48, 4 * 48], F32)
                    nc.vector.tensor_copy(gram_sb[:, 0:48], gfull[0:48, 0:48])
                    nc.vector.tensor_copy(gram_sb[:, 96:144], gfull[0:48, 96:144])
                    nc.sync.dma_start(gram_sb[:, 48:96], gfull[48:96, 48:96])
                    nc.sync.dma_start(gram_sb[:, 144:192], gfull[48:96, 144:192])
                    ssq3 = smallp.tile([128, 3], F32)
                    for t in range(3):
                        nc.vector.tensor_reduce(
                            ssq3[:, t : t + 1],
                            ssq_parts[:, NDW * t : NDW * (t + 1)], AX.X, OP.add)
                    nc.sync.dma_start(g_bin[:], gram_sb[:])
                    nc.sync.dma_start(s_bin[:], ssq3[:])
                    if no_cc:
                        nc.sync.dma_start(g_bout[:], g_bin[:])
                        nc.sync.dma_start(s_bout[:], s_bin[:])
                    else:
                        groups = [[0, 1], [2, 3], [4, 5], [6, 7]]
                        nc.gpsimd.collective_compute(
                            "AllReduce", OP.add, replica_groups=groups,
                            ins=[g_bin[:]], outs=[g_bout[:]])
                        nc.gpsimd.collective_compute(
                            "AllReduce", OP.add, replica_groups=groups,
                            ins=[s_bin[:]], outs=[s_bout[:]])
                    g2 = smallp.tile([48, 4 * 48], F32)
                    ssq_r = smallp.tile([128, 3], F32)
                    nc.sync.dma_start(g2[:], g_bout[:])
                    nc.sync.dma_start(ssq_r[:], s_bout[:])

                    nrm = smallp.tile([128, 3], F32)
                    nc.scalar.sqrt(nrm[:], ssq_r[:])
                    nc.vector.tensor_scalar_max(nrm[:], nrm[:], EPS)
                    rn = smallp.tile([128, 3], F32)
                    nc.vector.reciprocal(rn[:], nrm[:])
                    rn8 = smallp.tile([48, 8], F32)  # cols: q h0..h3, k h0..h3
                    for idx in range(8):
                        gch = 48 * idx  # q: 0..191, k: 192..383
                        t0, r0 = gch // 128, gch % 128
                        n1 = min(48, 128 - r0)
                        nc.sync.dma_start(rn8[0:n1, idx : idx + 1],
                                          rn[r0 : r0 + n1, t0 : t0 + 1])
                        if n1 < 48:
                            nc.sync.dma_start(rn8[n1:48, idx : idx + 1],
                                              rn[0 : 48 - n1, t0 + 1 : t0 + 2])

                    att = smallp.tile([48, 4 * 48], F32)
                    attT = smallp.tile([48, 4 * 48], F32)
                    mxs = smallp.tile([48, NH], F32)
                    sm = smallp.tile([48, NH], F32)
                    rs = smallp.tile([48, NH], F32)
                    for h in range(NH):
                        sl = slice(48 * h, 48 * h + 48)
                        # scale rows by rq[d]
                        nc.vector.tensor_scalar_mul(g2[:, sl], g2[:, sl],
                                                    rn8[:, h : h + 1])
                        # transpose, scale by rk[e], transpose back
                        tp = tps.tile([128, 128], F32, tag="tps")
                        nc.tensor.transpose(tp[0:48, 0:48], g2[:, sl],
                                            ident[0:48, 0:48])
                        gt = smallp.tile([48, 48], F32, tag="gt")
                        nc.scalar.activation(gt[:], tp[0:48, 0:48], AF.Identity,
                                             scale=rn8[:, 4 + h : 5 + h])
                        tp2 = tps.tile([128, 128], F32, tag="tps")
                        nc.tensor.transpose(tp2[0:48, 0:48], gt[:],
                                            ident[0:48, 0:48])
                        nc.vector.tensor_copy(g2[:, sl], tp2[0:48, 0:48])
                        # softmax over free dim with fused *SCALE
                        nc.vector.tensor_reduce(mxs[:, h : h + 1], g2[:, sl],
                                                AX.X, OP.max, negate=True)
                        nc.vector.tensor_scalar_mul(mxs[:, h : h + 1],
                                                    mxs[:, h : h + 1], SCALE)
                        nc.scalar.activation(att[:, sl], g2[:, sl], AF.Exp,
                                             bias=mxs[:, h : h + 1], scale=SCALE)
                        nc.vector.tensor_reduce(sm[:, h : h + 1], att[:, sl],
                                                AX.X, OP.add)
                        nc.vector.reciprocal(rs[:, h : h + 1], sm[:, h : h + 1])
                        nc.vector.tensor_scalar_mul(att[:, sl], att[:, sl],
                                                    rs[:, h : h + 1])
                        tp3 = tps.tile([128, 128], F32, tag="tps")
                        nc.tensor.transpose(tp3[0:48, 0:48], att[:, sl],
                                            ident[0:48, 0:48])
                        nc.vector.tensor_copy(attT[:, sl], tp3[0:48, 0:48])

                    blk01f = smallp.tile([96, 96], F32)
                    blk23f = smallp.tile([96, 96], F32)
                    nc.gpsimd.memset(blk01f[:], 0.0)
                    nc.gpsimd.memset(blk23f[:], 0.0)
                    nc.vector.tensor_copy(blk01f[0:48, 0:48], attT[:, 0:48])
                    nc.vector.tensor_copy(blk23f[0:48, 0:48], attT[:, 96:144])
                    nc.sync.dma_start(blk01f[48:96, 48:96], attT[:, 48:96])
                    nc.sync.dma_start(blk23f[48:96, 48:96], attT[:, 144:192])
                    blk01 = smallp.tile([96, 96], BF16)
                    blk23 = smallp.tile([96, 96], BF16)
                    nc.vector.tensor_copy(blk01[:], blk01f[:])
                    nc.vector.tensor_copy(blk23[:], blk23f[:])

                # ---- phase C: attn@v + proj ----
                with (
                    tc.tile_pool(name="aops", bufs=2, space="PSUM") as aops,
                    tc.tile_pool(name="prps", bufs=2, space="PSUM") as prps,
                ):
                    for i in range(NDW):
                        ao0 = aops.tile([96, 2, W], F32, tag="aops")
                        ao1 = aops.tile([96, 2, W], F32, tag="aops")
                        nc.tensor.matmul(ao0[:], blk01[:], vA[:, i],
                                         start=True, stop=True)
                        nc.tensor.matmul(ao1[:], blk23[:], vB[:, i],
                                         start=True, stop=True)
                        as0 = aosb.tile([96, 2, W], F32, tag="as0")
                        as1 = aosb.tile([96, 2, W], F32, tag="as1")
                        nc.vector.tensor_copy(r32(as0[:]), ao0[:])
                        nc.scalar.copy(r32(as1[:]), ao1[:])
                        for ot, (o0, no) in enumerate([(0, 128), (128, 64)]):
                            pp = prps.tile([128, 2, W], F32, tag="prps")
                            nc.tensor.matmul(pp[0:no], r32(wp_a[:, o0 : o0 + no]),
                                             r32(as0[:]), start=True, stop=False)
                            nc.tensor.matmul(pp[0:no], r32(wp_b[:, o0 : o0 + no]),
                                             r32(as1[:]), start=False, stop=True)
                            ob = outsbp.tile([128, 2, W], F32, tag=f"ob{ot}")
                            nc.scalar.activation(ob[0:no], pp[0:no], AF.Identity,
                                                 bias=bias_col(4, ot))
                            nc.sync.dma_start(
                                out_d[o0 : o0 + no, 2 * i : 2 * i + 2, :], ob[0:no])

    nc.compile()
    return nc


def _get_nc(repeat=1, no_cc=False):
    key = (repeat, no_cc)
    if key not in _CACHED:
        _CACHED[key] = _build_nc(repeat, no_cc)
    return _CACHED[key]


def _prep_inputs(x, w_qkv, b_qkv, w_dw, b_dw, w_proj, b_proj):
    x = np.asarray(x, np.float32)
    wq = np.asarray(w_qkv, np.float32)[:, :, 0, 0]        # [576, 192]
    bq = np.asarray(b_qkv, np.float32)
    wd = np.asarray(w_dw, np.float32)[:, 0]               # [576, 3, 3]
    bd = np.asarray(b_dw, np.float32)
    wp = np.asarray(w_proj, np.float32)[:, :, 0, 0]       # [192, 192]
    bp = np.asarray(b_proj, np.float32)

    wqkvT = np.ascontiguousarray(wq.T)                    # [192, 576]
    wprojT = np.ascontiguousarray(wp.T)                   # [192, 192]
    ident = np.eye(128, dtype=np.float32)
    wdw_diag = np.zeros((128, 9 * NT * 128), np.float32)
    for kh in range(3):
        for kw in range(3):
            for t, (c0, nt) in enumerate(CT):
                base = ((kh * 3 + kw) * NT + t) * 128
                idx = np.arange(nt)
                wdw_diag[idx, base + idx] = wd[c0 : c0 + nt, kh, kw]

    wdw_diag = wdw_diag.astype(NPBF16)

    def pack5(v):  # [576] -> [5, 128]
        o = np.zeros((5, 128), np.float32)
        for t, (c0, nt) in enumerate(CT):
            o[t, :nt] = v[c0 : c0 + nt]
        return o

    bq5, bd5 = pack5(bq), pack5(bd)
    zero5 = np.zeros((5, 128), np.float32)
    bp2 = np.zeros((2, 128), np.float32)
    bp2[0], bp2[1, :64] = bp[0:128], bp[128:192]

    xp = np.pad(x, ((0, 0), (0, 0), (1, 1), (0, 0)))      # [4, 192, 258, 256]
    in_maps = []
    for core in range(N_CORES):
        b, hf = divmod(core, 2)
        x_sh = np.ascontiguousarray(xp[b, :, hf * HALF : hf * HALF + PR, :])
        hb_top = zero5 if hf == 0 else bq5
        hb_bot = bq5 if hf == 0 else zero5
        bias_pack = np.concatenate([bq5, bd5, hb_top, hb_bot, bp2,
                                    pack5(wd[:, 0, 0]), pack5(wd[:, 0, 2])],
                                   axis=0)
        in_maps.append({
            "x_sh": x_sh, "wqkvT": wqkvT, "wdw_diag": wdw_diag,
            "wprojT": wprojT, "ident": ident,
            "bias_pack": np.ascontiguousarray(bias_pack),
        })
    return in_maps


def kernel(x, w_qkv, b_qkv, w_dw, b_dw, w_proj, b_proj):
    nc = _get_nc()
    in_maps = _prep_inputs(x, w_qkv, b_qkv, w_dw, b_dw, w_proj, b_proj)
    res = run_bass_kernel_spmd(nc, in_maps, core_ids=list(range(N_CORES)))
    out = np.empty((B, DIM, H, W), np.float32)
    for core in range(N_CORES):
        b, hf = divmod(core, 2)
        out[b, :, hf * HALF : (hf + 1) * HALF, :] = res.results[core]["out_sh"]
    return out

